# revision 1
# baseline (speedup 1.0000x reference)
"""GCN encoder (VGAE-style) distributed Bass kernel for 8 TRN2 NeuronCores.

Math restructure (exact up to float reassociation):
  A_hat = D^-1/2 (A + I) D^-1/2  with  D = indegree(A)+1  (self loops)
  hidden = softplus(A_hat @ (x @ W1) + b1)
  agg    = A_hat @ hidden            # shared by both heads (aggregation is linear)
  z_loc  = agg @ W_mu + b_mu
  z_scale= exp(agg @ W_sig + b_sig)

Sharding: nodes (dim 0) across 8 cores; edges bucketed by destination core so
the scatter-add stays local; weights replicated; all-gather of the
dinv-scaled bf16 feature tables between layers.

Device mapping per core:
  - scatter-add via one-hot matmuls: S[e,d] = (dst_local[e]==d), accumulated
    on PE into PSUM per 128-node window.  S is built on DVE (is_equal vs an
    iota row) or on ACT (Relu(1-|iota-dst|)) - the split matters because the
    Q7 descriptor generation of the gathers contends with DVE on the shared
    POOL/DVE SBUF port.
  - per-edge source rows fetched from the all-gathered bf16 table with
    dma_gather (int16 indices -> the table is processed in <=32k-row chunks;
    edges are bucketed (dst window x src chunk) and 128-padded per bucket).
    ~8.3ns/row of serial Q7 descriptor generation is the kernel's floor.
  - self loops never touch the gather: they are one identity-matmul per
    window from the SBUF-resident scaled features.
  - epilogues (softplus=ln(1+exp), exp, deg^-1/2=exp(-0.5 ln)) on ScalarE,
    batched across window groups; a single ACT function table
    (natural_log_exp_and_others) covers Abs/Relu/Identity/Exp/Ln so only one
    table load is ever emitted (the stock per-function table pick would
    otherwise reload on every Ln<->Exp switch).
"""

import sys

import numpy as np

sys.path.insert(0, "/opt/trn_rl_repo")

import concourse.bass as bass
import concourse.bacc as bacc
import concourse.mybir as mybir
import concourse.tile as tile
from concourse.bass_utils import run_bass_kernel_spmd
from concourse.library_config import mlp

NCORES = 8
P = 128            # partitions / window size
CHUNK_MAX = 32000  # dma_gather int16 index reach
GROUP = 4          # windows per gather group / epilogue batch

LAST_RESULT = None  # BassKernelResults of the most recent run (test harness)

_ACT_TABLE = "natural_log_exp_and_others"
_act_patched = False


def _patch_act_tables():
    """Force every activation onto the one table that contains all of
    Abs/Relu/Identity/Exp/Ln.  The stock pass assigns each function its
    first-containing table, which thrashes table loads (~1.3us each) on
    every Ln<->Exp switch.  Emptying the other tables (order preserved, so
    act_func_set_id still matches act_info.json) makes the pass emit exactly
    one load."""
    global _act_patched
    if _act_patched:
        return
    import concourse.hw_specs as hw_specs
    orig = hw_specs.get_activation_tables

    def patched(arch):
        tabs = orig(arch)
        return {name: (fns if name == _ACT_TABLE else set())
                for name, fns in tabs.items()}

    bacc.get_activation_tables = patched
    _act_patched = True


# --------------------------------------------------------------------------
# host-side sharding / layout prep
# --------------------------------------------------------------------------

def _host_prep(x, edge_index):
    n = x.shape[0]
    assert n % NCORES == 0
    nc_nodes = n // NCORES
    nw = (nc_nodes + P - 1) // P

    nch = -(-n // CHUNK_MAX)           # table chunks
    cs = -(-n // nch)                  # chunk size (rows)
    assert cs <= 32767

    src_a = np.asarray(edge_index[0]).astype(np.int64)
    dst_a = np.asarray(edge_index[1]).astype(np.int64)
    # self loops are handled as identity matmuls on-device, not as edges

    core = dst_a // nc_nodes
    rem = dst_a - core * nc_nodes
    win = rem >> 7
    dloc = (rem & 127).astype(np.float32)
    chunk = src_a // cs
    bucket = (core * nw + win) * nch + chunk

    nb = NCORES * nw * nch
    order = np.argsort(bucket, kind="stable")
    sb = (src_a - chunk * cs)[order].astype(np.int16)  # chunk-local src idx
    db = dloc[order]
    counts = np.bincount(bucket, minlength=nb).reshape(NCORES, nw, nch)
    bstart = np.zeros(nb + 1, np.int64)
    bstart[1:] = np.cumsum(counts.reshape(-1))

    nblk = -(-counts.max(axis=0) // P)  # [nw, nch] shared across cores
    groups = [list(range(g, min(g + GROUP, nw))) for g in range(0, nw, GROUP)]

    # layout walk: for each group, for each chunk, for each window in group
    calls = []        # per group: list of (k, num_idxs, idx_col0, blk0)
    wblocks = [[] for _ in range(nw)]  # per window: (groupblk_pos, dstl_col)
    idx_cols = 0
    dstl_cols = 0
    seq = []          # flat walk: (w, k, nblk) in layout order
    gmaxblk = 0
    for g in groups:
        gb = 0
        gcalls = []
        for k in range(nch):
            tot = int(sum(nblk[w, k] for w in g))
            if tot == 0:
                continue
            gcalls.append((k, tot * P, idx_cols, gb))
            for w in g:
                nbk = int(nblk[w, k])
                if nbk == 0:
                    continue
                for b in range(nbk):
                    wblocks[w].append((gb + b, dstl_cols + b))
                seq.append((w, k, nbk))
                idx_cols += nbk * 8   # 128 idx per block / 16 rows
                dstl_cols += nbk
                gb += nbk
        calls.append(gcalls)
        gmaxblk = max(gmaxblk, gb)

    ltot = max(idx_cols, 1)
    btot = max(dstl_cols, 1)
    src16 = np.zeros((NCORES, P, ltot), np.int16)
    dstl = np.full((NCORES, P, btot), -1.0, np.float32)
    for c in range(NCORES):
        icol = 0
        dcol = 0
        for (w, k, nbk) in seq:
            b = (c * nw + w) * nch + k
            s, e = bstart[b], bstart[b + 1]
            cnt = int(e - s)
            pad = nbk * P
            sl = np.zeros(pad, np.int16)
            dl = np.full(pad, -1.0, np.float32)
            sl[:cnt] = sb[s:e]
            dl[:cnt] = db[s:e]
            # stream position i -> idx16[i%16, i//16]; replicated x8 rows
            src16[c][:, icol:icol + nbk * 8] = np.tile(
                sl.reshape(nbk * 8, 16).T, (8, 1))
            # edge i -> msg[partition i%128, block i//128]
            dstl[c][:, dcol:dcol + nbk] = dl.reshape(nbk, P).T
            icol += nbk * 8
            dcol += nbk
    return {
        "n": n, "nc_nodes": nc_nodes, "nw": nw, "nch": nch, "cs": cs,
        "groups": groups, "calls": calls, "wblocks": wblocks,
        "gmaxblk": gmaxblk, "ltot": ltot, "btot": btot,
        "src16": src16, "dstl": dstl,
    }


# --------------------------------------------------------------------------
# device program
# --------------------------------------------------------------------------

def _build(dims, fdim, hdim, zdim, debug=False):
    """Build the SPMD Bass program (identical for all 8 cores)."""
    n = dims["n"]
    nc_nodes = dims["nc_nodes"]
    nw = dims["nw"]
    nch = dims["nch"]
    cs = dims["cs"]
    groups = dims["groups"]
    calls = dims["calls"]
    wblocks = dims["wblocks"]
    gmaxblk = dims["gmaxblk"]
    ltot = dims["ltot"]
    btot = dims["btot"]
    fk = fdim // P
    zc = 2 * zdim
    assert hdim == P and zc == P

    f32 = mybir.dt.float32
    bf16 = mybir.dt.bfloat16
    i16 = mybir.dt.int16
    AF = mybir.ActivationFunctionType
    OP = mybir.AluOpType
    rg = [list(range(NCORES))]

    _patch_act_tables()
    nc = bacc.Bacc("TRN2", target_bir_lowering=False, debug=False,
                   num_devices=NCORES)

    # ---- kernel I/O ----
    xT_d = nc.dram_tensor("xT", [fdim, nc_nodes], f32, kind="ExternalInput")
    w1_d = nc.dram_tensor("W1", [fdim, hdim], f32, kind="ExternalInput")
    wc_d = nc.dram_tensor("Wcat", [hdim, zc], f32, kind="ExternalInput")
    b1_d = nc.dram_tensor("b1bc", [P, GROUP * hdim], f32, kind="ExternalInput")
    bc_d = nc.dram_tensor("bcat", [zc, 1], f32, kind="ExternalInput")
    io_d = nc.dram_tensor("iota", [P, P], f32, kind="ExternalInput")
    id_d = nc.dram_tensor("ident", [P, P], f32, kind="ExternalInput")
    src_d = nc.dram_tensor("srcidx", [P, ltot], i16, kind="ExternalInput")
    dst_d = nc.dram_tensor("dstloc", [P, btot], f32, kind="ExternalInput")
    nds_d = nc.dram_tensor("ndstloc", [P, btot], f32, kind="ExternalInput")
    out_d = nc.dram_tensor("z_out", [zc, nc_nodes], f32, kind="ExternalOutput")
    if debug:
        dbg_dinv = nc.dram_tensor("dbg_dinv", [P, nw], f32,
                                  kind="ExternalOutput")
        dbg_ag1 = nc.dram_tensor("dbg_ag1", [nc_nodes, hdim], bf16,
                                 kind="ExternalOutput")
        dbg_tbl = nc.dram_tensor("dbg_tbl", [n, hdim], bf16,
                                 kind="ExternalOutput")
        dbg_ag2 = nc.dram_tensor("dbg_ag2", [nc_nodes, hdim], bf16,
                                 kind="ExternalOutput")

    def rows_of(w):
        return min(P, nc_nodes - w * P)

    with tile.TileContext(nc) as tc:
        with (
            tc.tile_pool(name="const", bufs=1) as cpool,
            tc.tile_pool(name="idx", bufs=1) as ipool,
            tc.tile_pool(name="keep", bufs=1) as kpool,
            tc.tile_pool(name="psio", bufs=1, space="PSUM") as psio,
            tc.tile_pool(name="dram", bufs=1, space="DRAM") as dpool,
        ):
            nc.gpsimd.load_library(mlp)

            iota_bf = cpool.tile([P, P], bf16)
            ident_bf = cpool.tile([P, P], bf16)
            w1_bf = cpool.tile([P, fk * hdim], bf16, name="w1bf")
            wc_bf = cpool.tile([P, zc], bf16)
            b1_sb = cpool.tile([P, GROUP * hdim], f32)
            bc_sb = cpool.tile([zc, 1], f32)
            dinv_sb = cpool.tile([P, nw], f32)
            deg_sb = cpool.tile([P, nw], f32)

            src_sb = ipool.tile([P, ltot], i16)
            dst_sb = ipool.tile([P, btot], f32)
            nds_sb = ipool.tile([P, btot], f32)
            # dinv-scaled own rows (self-loop source): own2 = hidden' layer
            own2_sb = kpool.tile([P, nw * hdim], bf16, name="own2")
            nc.vector.memset(own2_sb[:], 0.0)

            nc.gpsimd.dma_start(out=iota_bf[:], in_=io_d[:, :])
            nc.gpsimd.dma_start(out=ident_bf[:], in_=id_d[:, :])
            for k in range(fk):
                nc.gpsimd.dma_start(
                    out=w1_bf[:, k * hdim:(k + 1) * hdim],
                    in_=w1_d[k * P:(k + 1) * P, :])
            nc.gpsimd.dma_start(out=wc_bf[:], in_=wc_d[:, :])
            nc.sync.dma_start(out=b1_sb[:], in_=b1_d[:, :])
            nc.sync.dma_start(out=bc_sb[:], in_=bc_d[:, :])
            nc.sync.dma_start(out=src_sb[:], in_=src_d[:, :])
            nc.sync.dma_start(out=dst_sb[:], in_=dst_d[:, :])
            nc.sync.dma_start(out=nds_sb[:], in_=nds_d[:, :])

            ag1_in = dpool.tile([nc_nodes, hdim], bf16)
            ag2_in = dpool.tile([nc_nodes, hdim], bf16)
            h1_tbl = dpool.tile([n, hdim], bf16, addr_space="Shared")
            h2_tbl = dpool.tile([n, hdim], bf16, addr_space="Shared")

            # own1 = h1' layer self-loop source; lives until end of phase B
            keepA = tc.tile_pool(name="keepA", bufs=1)
            keepA_pool = keepA.__enter__()
            own_sb = keepA_pool.tile([P, nw * hdim], bf16, name="own1")
            nc.vector.memset(own_sb[:], 0.0)

            # PSUM-resident iota: a PSUM source caps DVE at single-read-port
            # mode, which keeps is_equal off the SBUF port pair that the Q7
            # gather descriptor generation contends on.
            iota_ps = psio.tile([P, P], f32, name="iops")
            nc.vector.tensor_copy(iota_ps[:], iota_bf[:])

            # S-tile builders.  DVE: one is_equal.  ACT: Relu(1-|iota-dst|)
            # (exact for integer-valued inputs).  dcol indexes dst_sb/nds_sb.
            def build_s(spool, tpool, dcol, on_act):
                s_t = spool.tile([P, P], bf16, tag="s")
                if on_act:
                    u_t = tpool.tile([P, P], bf16, tag="u")
                    nc.scalar.activation(u_t[:], iota_bf[:], AF.Abs,
                                         bias=nds_sb[:, dcol:dcol + 1])
                    nc.scalar.activation(s_t[:], u_t[:], AF.Relu,
                                         bias=1.0, scale=-1.0)
                else:
                    nc.vector.tensor_scalar(
                        out=s_t[:], in0=iota_ps[:],
                        scalar1=dst_sb[:, dcol:dcol + 1], scalar2=None,
                        op0=OP.is_equal)
                return s_t

            # ---------- phase A: degree histogram + x @ W1 (+ dinv scale) ----
            with (
                tc.tile_pool(name="xk", bufs=1) as xpool,
                tc.tile_pool(name="sA", bufs=6) as spool,
                tc.tile_pool(name="eA", bufs=3) as epool,
                tc.tile_pool(name="psdeg", bufs=2, space="PSUM") as pdeg,
                tc.tile_pool(name="psh1", bufs=2, space="PSUM") as ph1,
            ):
                ones_bf = cpool.tile([P, 1], bf16)
                nc.vector.memset(ones_bf[:], 1.0)
                xk_sb = []
                for k in range(fk):
                    xk = xpool.tile([P, nc_nodes], bf16, name=f"xk{k}")
                    nc.gpsimd.dma_start(out=xk[:], in_=xT_d[k * P:(k + 1) * P, :])
                    xk_sb.append(xk)

                # deg histogram; S-builds split DVE/ACT (POOL is idle here so
                # DVE runs at full speed - give DVE the larger share)
                for w in range(nw):
                    blocks = wblocks[w]
                    degp = pdeg.tile([P, 1], f32, tag="deg")
                    for j, (_, dcol) in enumerate(blocks):
                        s_t = build_s(spool, spool, dcol, on_act=(j % 3 == 2))
                        nc.tensor.matmul(degp[:], s_t[:], ones_bf[:],
                                         start=(j == 0),
                                         stop=(j == len(blocks) - 1))
                    # deg = indeg + 1 (self loop); also covers padded rows
                    nc.vector.tensor_scalar(out=deg_sb[:, w:w + 1],
                                            in0=degp[:], scalar1=1.0,
                                            scalar2=None, op0=OP.add)
                # dinv = exp(-0.5*ln(deg)), batched over all windows
                dln = epool.tile([P, nw], f32, tag="dln")
                nc.scalar.activation(dln[:], deg_sb[:], AF.Ln)
                nc.scalar.activation(dinv_sb[:], dln[:], AF.Exp, scale=-0.5)

                # h1' = dinv * (x @ W1); kept in SBUF (self loops) and sent
                # to the all-gather bounce
                for w in range(nw):
                    r = rows_of(w)
                    h1p = ph1.tile([P, hdim], f32, tag="h1")
                    for k in range(fk):
                        nc.tensor.matmul(
                            h1p[:r, :],
                            xk_sb[k][:, w * P:w * P + r],
                            w1_bf[:, k * hdim:(k + 1) * hdim],
                            start=(k == 0), stop=(k == fk - 1))
                    nc.vector.tensor_scalar(
                        out=own_sb[:r, w * hdim:(w + 1) * hdim],
                        in0=h1p[:r, :],
                        scalar1=dinv_sb[:r, w:w + 1], scalar2=None,
                        op0=OP.mult)
                    nc.sync.dma_start(
                        out=ag1_in[w * P:w * P + r, :],
                        in_=own_sb[:r, w * hdim:w * hdim + hdim])

            nc.gpsimd.collective_compute(
                "AllGather", OP.bypass, replica_groups=rg,
                ins=[ag1_in.opt()], outs=[h1_tbl.opt()])
            if debug:
                nc.sync.dma_start(out=dbg_ag1[:, :], in_=ag1_in[:, :])
                nc.sync.dma_start(out=dbg_tbl[:, :], in_=h1_tbl[:, :])

            # ---------- aggregation pass helper ----------
            # Gathers + S-matmuls for one pass; epilogue(group, accs) runs
            # once per group with the list of (w, acc_psum).
            def agg_pass(tbl, mpool, spool, epool, pacc, epilogue, mbufs,
                         own, act_share=True):
                gi = 0
                for g, gcalls in zip(groups, calls):
                    msg = mpool.tile([P, gmaxblk, P], bf16, tag="msg")
                    if gi < mbufs:  # stale-SBUF NaN guard on fresh slots
                        nc.vector.memset(msg[:], 0.0)
                    for (k, nidx, icol, blk0) in gcalls:
                        csz = min(cs, n - k * cs)
                        nc.gpsimd.dma_gather(
                            msg[:, blk0:blk0 + nidx // P, :],
                            tbl[k * cs:k * cs + csz, :],
                            src_sb[:, icol:icol + nidx // 16],
                            nidx, nidx, hdim, single_packet=False)
                    accs = []
                    for w in g:
                        blocks = wblocks[w]
                        acc = pacc.tile([P, hdim], f32, tag="acc")
                        # self loop: + own window rows (identity one-hot)
                        nc.tensor.matmul(acc[:], ident_bf[:],
                                         own[:, w * hdim:(w + 1) * hdim],
                                         start=True,
                                         stop=(len(blocks) == 0))
                        for j, (bpos, dcol) in enumerate(blocks):
                            # gathers keep POOL busy; POOL contends with DVE
                            # on the shared SBUF port - run 2/3 of the
                            # builds on ACT here
                            s_t = build_s(spool, spool, dcol,
                                          on_act=act_share and (j % 3 != 2))
                            nc.tensor.matmul(acc[:], s_t[:],
                                             msg[:, bpos, :],
                                             start=False,
                                             stop=(j == len(blocks) - 1))
                        accs.append((w, acc))
                    epilogue(g, accs)
                    gi += 1

            # ---------- phase B: aggregate layer 1, softplus, rescale --------
            with (
                tc.tile_pool(name="msgB", bufs=2) as mpool,
                tc.tile_pool(name="sB", bufs=6) as spool,
                tc.tile_pool(name="eB", bufs=3) as epool,
                tc.tile_pool(name="psB", bufs=4, space="PSUM") as pacc,
            ):
                def epi1(g, accs):
                    gw = len(accs)
                    stage = epool.tile([P, GROUP * hdim], f32, tag="st")
                    for i, (w, acc) in enumerate(accs):
                        # dinv*acc, PSUM -> staging (ACT, per window)
                        nc.scalar.activation(
                            stage[:, i * hdim:(i + 1) * hdim], acc[:],
                            AF.Identity, scale=dinv_sb[:, w:w + 1])
                    sl = stage[:, :gw * hdim]
                    nc.vector.tensor_add(sl, sl, b1_sb[:, :gw * hdim])
                    # softplus(x) = ln(exp(x) + 1), batched over the group;
                    # the +1 folds into Ln's bias
                    ex = epool.tile([P, GROUP * hdim], f32, tag="ex")
                    nc.scalar.activation(ex[:, :gw * hdim], sl, AF.Exp)
                    nc.scalar.activation(sl, ex[:, :gw * hdim], AF.Ln,
                                         bias=1.0)
                    for i, (w, acc) in enumerate(accs):
                        r = rows_of(w)
                        # hidden' = dinv*softplus; resident + all-gather copy
                        nc.scalar.activation(
                            own2_sb[:, w * hdim:(w + 1) * hdim],
                            stage[:, i * hdim:(i + 1) * hdim],
                            AF.Identity, scale=dinv_sb[:, w:w + 1])
                        nc.sync.dma_start(
                            out=ag2_in[w * P:w * P + r, :],
                            in_=own2_sb[:r, w * hdim:(w + 1) * hdim])

                agg_pass(h1_tbl, mpool, spool, epool, pacc, epi1, 2, own_sb)
            keepA.__exit__(None, None, None)

            nc.gpsimd.collective_compute(
                "AllGather", OP.bypass, replica_groups=rg,
                ins=[ag2_in.opt()], outs=[h2_tbl.opt()])
            if debug:
                nc.sync.dma_start(out=dbg_ag2[:, :], in_=ag2_in[:, :])
                nc.sync.dma_start(out=dbg_dinv[:, :], in_=dinv_sb[:])

            # ---------- phase C: aggregate layer 2, output heads -------------
            with (
                tc.tile_pool(name="zT", bufs=1) as zpool,
                tc.tile_pool(name="msgC", bufs=2) as mpool,
                tc.tile_pool(name="sC", bufs=6) as spool,
                tc.tile_pool(name="eC", bufs=4) as epool,
                tc.tile_pool(name="psC", bufs=3, space="PSUM") as pacc,
                tc.tile_pool(name="psT", bufs=2, space="PSUM") as ptr,
            ):
                zt_sb = zpool.tile([zc, nc_nodes], f32)

                def epi2(g, accs):
                    for i, (w, acc) in enumerate(accs):
                        r = rows_of(w)
                        a2bf = epool.tile([P, hdim], bf16, tag="a2bf")
                        nc.scalar.activation(a2bf[:], acc[:], AF.Identity,
                                             scale=dinv_sb[:, w:w + 1])
                        tp = ptr.tile([P, P], bf16, tag="tp")
                        nc.tensor.transpose(tp[:], a2bf[:], ident_bf[:])
                        a2t = epool.tile([P, P], bf16, tag="a2t")
                        nc.scalar.activation(a2t[:], tp[:], AF.Identity)
                        ztp = ptr.tile([zc, P], f32, tag="ztp")
                        nc.tensor.matmul(ztp[:], wc_bf[:], a2t[:],
                                         start=True, stop=True)
                        nc.scalar.activation(
                            zt_sb[0:zdim, w * P:w * P + r], ztp[0:zdim, :r],
                            AF.Identity, bias=bc_sb[0:zdim, 0:1])
                        nc.scalar.activation(
                            zt_sb[zdim:zc, w * P:w * P + r], ztp[zdim:zc, :r],
                            AF.Exp, bias=bc_sb[zdim:zc, 0:1])

                agg_pass(h2_tbl, mpool, spool, epool, pacc, epi2, 2, own2_sb)
                nc.sync.dma_start(out=out_d[:, :], in_=zt_sb[:])

    nc.compile()
    return nc


# --------------------------------------------------------------------------
# entry point
# --------------------------------------------------------------------------

def _install_profile_hook():
    """The agent image's antenv lacks axon_hooks; recreate it from the
    boot helpers so trace=True can capture NTFF exec times."""
    try:
        import antenv.axon_hooks  # noqa: F401
        return
    except ImportError:
        pass
    try:
        import types
        if "/root/.axon_site" not in sys.path:
            sys.path.insert(0, "/root/.axon_site")
        from trn_agent_boot.trn_boot import _ntff_profile_via_ctypes
        hook = _ntff_profile_via_ctypes("/opt/axon/libaxon_pjrt.so")
        mod = types.ModuleType("antenv.axon_hooks")
        mod.get_axon_ntff_profile_hook = lambda: hook
        mod.set_axon_ntff_profile_hook = lambda h: None
        sys.modules["antenv.axon_hooks"] = mod
    except Exception:
        pass


def kernel(x, edge_index, W1, b1, W_mu, b_mu, W_sig, b_sig, _trace=False,
           _debug=False):
    global LAST_RESULT
    if _trace:
        _install_profile_hook()
    x = np.ascontiguousarray(np.asarray(x, dtype=np.float32))
    W1 = np.asarray(W1, dtype=np.float32)
    b1 = np.asarray(b1, dtype=np.float32)
    W_mu = np.asarray(W_mu, dtype=np.float32)
    b_mu = np.asarray(b_mu, dtype=np.float32)
    W_sig = np.asarray(W_sig, dtype=np.float32)
    b_sig = np.asarray(b_sig, dtype=np.float32)

    n, fdim = x.shape
    hdim = W1.shape[1]
    zdim = W_mu.shape[1]
    dims = _host_prep(x, edge_index)
    nc_nodes = dims["nc_nodes"]

    nc = _build(dims, fdim, hdim, zdim, debug=_debug)

    wcat = np.ascontiguousarray(np.concatenate([W_mu, W_sig], axis=1))
    b1bc = np.ascontiguousarray(
        np.tile(b1[None, :], (P, GROUP)))
    bcat = np.ascontiguousarray(
        np.concatenate([b_mu, b_sig]).reshape(2 * zdim, 1))
    iota = np.ascontiguousarray(
        np.tile(np.arange(P, dtype=np.float32)[None, :], (P, 1)))
    ident = np.eye(P, dtype=np.float32)

    in_maps = []
    for c in range(NCORES):
        xt_c = np.ascontiguousarray(
            x[c * nc_nodes:(c + 1) * nc_nodes, :].T)
        in_maps.append({
            "xT": xt_c,
            "W1": W1, "Wcat": wcat, "b1bc": b1bc, "bcat": bcat,
            "iota": iota, "ident": ident,
            "srcidx": dims["src16"][c], "dstloc": dims["dstl"][c],
            "ndstloc": np.ascontiguousarray(-dims["dstl"][c]),
        })

    res = run_bass_kernel_spmd(nc, in_maps, core_ids=list(range(NCORES)),
                               trace=_trace)
    LAST_RESULT = res

    z = np.concatenate([res.results[c]["z_out"] for c in range(NCORES)],
                       axis=1)  # [2Z, N]
    z_loc = np.ascontiguousarray(z[:zdim, :].T)
    z_scale = np.ascontiguousarray(z[zdim:, :].T)
    return z_loc, z_scale



# revision 10
# speedup vs baseline: 1.8812x; 1.8812x over previous
"""GCN encoder (VGAE-style) distributed Bass kernel for 8 TRN2 NeuronCores.

Math restructure (exact up to float reassociation):
  A_hat = D^-1/2 (A + I) D^-1/2  with  D = indegree(A)+1  (self loops)
  hidden = softplus(A_hat @ (x @ W1) + b1)
  agg    = A_hat @ hidden            # shared by both heads (aggregation is linear)
  z_loc  = agg @ W_mu + b_mu
  z_scale= exp(agg @ W_sig + b_sig)

Sharding: nodes (dim 0) across 8 cores; edges bucketed by destination core so
the scatter-add stays local; weights replicated; all-gather of the
dinv-scaled bf16 feature tables between layers.

Device mapping per core:
  - scatter-add via one-hot matmuls: S[e,d] = (dst_local[e]==d), accumulated
    on PE into PSUM per 128-node window.  S is built on DVE (is_equal vs an
    iota row) or on ACT (Relu(1-|iota-dst|)) - the split matters because the
    Q7 descriptor generation of the gathers contends with DVE on the shared
    POOL/DVE SBUF port.
  - per-edge source rows fetched from the all-gathered bf16 table with
    dma_gather (int16 indices -> the table is processed in <=32k-row chunks;
    edges are bucketed (dst window x src chunk) and 128-padded per bucket).
    ~8.3ns/row of serial Q7 descriptor generation is the kernel's floor.
  - self loops never touch the gather: they are one identity-matmul per
    window from the SBUF-resident scaled features.
  - epilogues (softplus=ln(1+exp), exp, deg^-1/2=exp(-0.5 ln)) on ScalarE,
    batched across window groups; a single ACT function table
    (natural_log_exp_and_others) covers Abs/Relu/Identity/Exp/Ln so only one
    table load is ever emitted (the stock per-function table pick would
    otherwise reload on every Ln<->Exp switch).
"""

import sys

import numpy as np

sys.path.insert(0, "/opt/trn_rl_repo")

import concourse.bass as bass
import concourse.bacc as bacc
import concourse.mybir as mybir
import concourse.tile as tile
from concourse.bass_utils import run_bass_kernel_spmd
from concourse.library_config import mlp

NCORES = 8
P = 128            # partitions / window size
CHUNK_MAX = 32000  # dma_gather int16 index reach
GROUP = 4          # windows per gather group / epilogue batch

LAST_RESULT = None  # BassKernelResults of the most recent run (test harness)

_ACT_TABLE = "natural_log_exp_and_others"
_act_patched = False


def _patch_act_tables():
    """Force every activation onto the one table that contains all of
    Abs/Relu/Identity/Exp/Ln.  The stock pass assigns each function its
    first-containing table, which thrashes table loads (~1.3us each) on
    every Ln<->Exp switch.  Emptying the other tables (order preserved, so
    act_func_set_id still matches act_info.json) makes the pass emit exactly
    one load."""
    global _act_patched
    if _act_patched:
        return
    import concourse.hw_specs as hw_specs
    orig = hw_specs.get_activation_tables

    def patched(arch):
        tabs = orig(arch)
        return {name: (fns if name == _ACT_TABLE else set())
                for name, fns in tabs.items()}

    bacc.get_activation_tables = patched
    _act_patched = True


# --------------------------------------------------------------------------
# host-side sharding / layout prep
# --------------------------------------------------------------------------

def _host_prep(x, edge_index):
    n = x.shape[0]
    assert n % NCORES == 0
    nc_nodes = n // NCORES
    nw = (nc_nodes + P - 1) // P

    nch = -(-n // CHUNK_MAX)           # table chunks
    cs = -(-n // nch)                  # chunk size (rows)
    assert cs <= 32767

    src_a = np.asarray(edge_index[0]).astype(np.int64)
    dst_a = np.asarray(edge_index[1]).astype(np.int64)
    # self loops are handled as identity matmuls on-device, not as edges

    core = dst_a // nc_nodes
    rem = dst_a - core * nc_nodes
    win = rem >> 7
    dloc = (rem & 127).astype(np.float32)
    chunk = src_a // cs
    bucket = (core * nw + win) * nch + chunk

    nb = NCORES * nw * nch
    order = np.argsort(bucket, kind="stable")
    sb = (src_a - chunk * cs)[order].astype(np.int16)  # chunk-local src idx
    db = dloc[order]
    counts = np.bincount(bucket, minlength=nb).reshape(NCORES, nw, nch)
    bstart = np.zeros(nb + 1, np.int64)
    bstart[1:] = np.cumsum(counts.reshape(-1))

    nblk = -(-counts.max(axis=0) // P)  # [nw, nch] shared across cores
    groups = [list(range(g, min(g + GROUP, nw))) for g in range(0, nw, GROUP)]

    # dinv = (indeg+1)^-1/2 — graph-structure metadata, host-computed like
    # the edge bucketing.  [NCORES, P, nw]: partition = row-in-window.
    deg = np.bincount(dst_a, minlength=n).astype(np.float32) + 1.0
    dinv_full = 1.0 / np.sqrt(deg)
    dinvt = np.ones((NCORES, nw * P), np.float32)
    for c in range(NCORES):
        dinvt[c, :nc_nodes] = dinv_full[c * nc_nodes:(c + 1) * nc_nodes]
    dinvt = np.ascontiguousarray(
        dinvt.reshape(NCORES, nw, P).transpose(0, 2, 1))

    # layout walk: for each group, for each chunk, for each window in group
    calls = []        # per group: list of (k, num_idxs, idx_col0, blk0)
    wblocks = [[] for _ in range(nw)]  # per window: (groupblk_pos, dstl_col)
    idx_cols = 0
    dstl_cols = 0
    seq = []          # flat walk: (w, k, nblk) in layout order
    gmaxblk = 0
    for g in groups:
        gb = 0
        gcalls = []
        for k in range(nch):
            tot = int(sum(nblk[w, k] for w in g))
            if tot == 0:
                continue
            gcalls.append((k, tot * P, idx_cols, gb))
            for w in g:
                nbk = int(nblk[w, k])
                if nbk == 0:
                    continue
                for b in range(nbk):
                    wblocks[w].append((gb + b, dstl_cols + b))
                seq.append((w, k, nbk))
                idx_cols += nbk * 8   # 128 idx per block / 16 rows
                dstl_cols += nbk
                gb += nbk
        calls.append(gcalls)
        gmaxblk = max(gmaxblk, gb)

    ltot = max(idx_cols, 1)
    btot = max(dstl_cols, 1)
    src16 = np.zeros((NCORES, P, ltot), np.int16)
    dstl = np.full((NCORES, P, btot), -1.0, np.float32)
    for c in range(NCORES):
        icol = 0
        dcol = 0
        for (w, k, nbk) in seq:
            b = (c * nw + w) * nch + k
            s, e = bstart[b], bstart[b + 1]
            cnt = int(e - s)
            pad = nbk * P
            sl = np.zeros(pad, np.int16)
            dl = np.full(pad, -1.0, np.float32)
            sl[:cnt] = sb[s:e]
            dl[:cnt] = db[s:e]
            # stream position i -> idx16[i%16, i//16]; replicated x8 rows
            src16[c][:, icol:icol + nbk * 8] = np.tile(
                sl.reshape(nbk * 8, 16).T, (8, 1))
            # edge i -> msg[partition i%128, block i//128]
            dstl[c][:, dcol:dcol + nbk] = dl.reshape(nbk, P).T
            icol += nbk * 8
            dcol += nbk
    return {
        "n": n, "nc_nodes": nc_nodes, "nw": nw, "nch": nch, "cs": cs,
        "groups": groups, "calls": calls, "wblocks": wblocks,
        "gmaxblk": gmaxblk, "ltot": ltot, "btot": btot,
        "src16": src16, "dstl": dstl, "dinvt": dinvt,
    }


# --------------------------------------------------------------------------
# device program
# --------------------------------------------------------------------------

def _build(dims, fdim, hdim, zdim, debug=False):
    """Build the SPMD Bass program (identical for all 8 cores)."""
    n = dims["n"]
    nc_nodes = dims["nc_nodes"]
    nw = dims["nw"]
    nch = dims["nch"]
    cs = dims["cs"]
    groups = dims["groups"]
    calls = dims["calls"]
    wblocks = dims["wblocks"]
    gmaxblk = dims["gmaxblk"]
    ltot = dims["ltot"]
    btot = dims["btot"]
    fk = fdim // P
    zc = 2 * zdim
    assert hdim == P and zc == P

    f32 = mybir.dt.float32
    bf16 = mybir.dt.bfloat16
    i16 = mybir.dt.int16
    AF = mybir.ActivationFunctionType
    OP = mybir.AluOpType
    rg = [list(range(NCORES))]

    _patch_act_tables()
    nc = bacc.Bacc("TRN2", target_bir_lowering=False, debug=False,
                   num_devices=NCORES, num_swdge_queues=4)

    # ---- kernel I/O ----
    xT_d = nc.dram_tensor("xT", [fdim, nc_nodes], f32, kind="ExternalInput")
    w1_d = nc.dram_tensor("W1", [fdim, hdim], f32, kind="ExternalInput")
    wc_d = nc.dram_tensor("Wcat", [hdim, zc], f32, kind="ExternalInput")
    b1_d = nc.dram_tensor("b1bc", [P, GROUP * hdim], f32, kind="ExternalInput")
    bc_d = nc.dram_tensor("bcat", [zc, 1], f32, kind="ExternalInput")
    io_d = nc.dram_tensor("iota", [P, P], f32, kind="ExternalInput")
    id_d = nc.dram_tensor("ident", [P, P], f32, kind="ExternalInput")
    src_d = nc.dram_tensor("srcidx", [P, ltot], i16, kind="ExternalInput")
    dst_d = nc.dram_tensor("dstloc", [P, btot], f32, kind="ExternalInput")
    nds_d = nc.dram_tensor("ndstloc", [P, btot], f32, kind="ExternalInput")
    dnv_d = nc.dram_tensor("dinvt", [P, nw], f32, kind="ExternalInput")
    out_d = nc.dram_tensor("z_out", [zc, nc_nodes], f32, kind="ExternalOutput")
    if debug:
        dbg_dinv = nc.dram_tensor("dbg_dinv", [P, nw], f32,
                                  kind="ExternalOutput")
        dbg_ag1 = nc.dram_tensor("dbg_ag1", [nc_nodes, hdim], bf16,
                                 kind="ExternalOutput")
        dbg_tbl = nc.dram_tensor("dbg_tbl", [n, hdim], bf16,
                                 kind="ExternalOutput")
        dbg_ag2 = nc.dram_tensor("dbg_ag2", [nc_nodes, hdim], bf16,
                                 kind="ExternalOutput")

    def rows_of(w):
        return min(P, nc_nodes - w * P)

    with tile.TileContext(nc) as tc:
        with (
            tc.tile_pool(name="const", bufs=1) as cpool,
            tc.tile_pool(name="idx", bufs=1) as ipool,
            tc.tile_pool(name="keep", bufs=1) as kpool,
            tc.tile_pool(name="psio", bufs=1, space="PSUM") as psio,
            tc.tile_pool(name="dram", bufs=1, space="DRAM") as dpool,
        ):
            nc.gpsimd.load_library(mlp)

            iota_bf = cpool.tile([P, P], bf16)
            ident_bf = cpool.tile([P, P], bf16)
            w1_bf = cpool.tile([P, fk * hdim], bf16, name="w1bf")
            wc_bf = cpool.tile([P, zc], bf16)
            b1_sb = cpool.tile([P, GROUP * hdim], f32)
            bc_sb = cpool.tile([zc, 1], f32)
            dinv_sb = cpool.tile([P, nw], f32)

            src_sb = ipool.tile([P, ltot], i16)
            dst_sb = ipool.tile([P, btot], f32)
            nds_sb = ipool.tile([P, btot], f32)
            # dinv-scaled own rows (self-loop source): own2 = hidden' layer
            own2_sb = kpool.tile([P, nw * hdim], bf16, name="own2")
            nc.vector.memset(own2_sb[:], 0.0)

            nc.gpsimd.dma_start(out=iota_bf[:], in_=io_d[:, :])
            nc.gpsimd.dma_start(out=ident_bf[:], in_=id_d[:, :])
            for k in range(fk):
                nc.gpsimd.dma_start(
                    out=w1_bf[:, k * hdim:(k + 1) * hdim],
                    in_=w1_d[k * P:(k + 1) * P, :])
            nc.gpsimd.dma_start(out=wc_bf[:], in_=wc_d[:, :])
            nc.sync.dma_start(out=b1_sb[:], in_=b1_d[:, :])
            nc.sync.dma_start(out=bc_sb[:], in_=bc_d[:, :])
            nc.sync.dma_start(out=src_sb[:], in_=src_d[:, :])
            nc.sync.dma_start(out=dst_sb[:], in_=dst_d[:, :])
            nc.sync.dma_start(out=nds_sb[:], in_=nds_d[:, :])
            nc.sync.dma_start(out=dinv_sb[:], in_=dnv_d[:, :])

            ag1_in = dpool.tile([nc_nodes, hdim], bf16)
            ag2_in = dpool.tile([nc_nodes, hdim], bf16)
            h1_tbl = dpool.tile([n, hdim], bf16, addr_space="Shared")
            h2_tbl = dpool.tile([n, hdim], bf16, addr_space="Shared")

            # own1 = h1' layer self-loop source; lives until end of phase B
            keepA = tc.tile_pool(name="keepA", bufs=1)
            keepA_pool = keepA.__enter__()
            own_sb = keepA_pool.tile([P, nw * hdim], bf16, name="own1")
            nc.vector.memset(own_sb[:], 0.0)

            # PSUM-resident iota: a PSUM source caps DVE at single-read-port
            # mode, which keeps is_equal off the SBUF port pair that the Q7
            # gather descriptor generation contends on.
            iota_ps = psio.tile([P, P], f32, name="iops")
            nc.vector.tensor_copy(iota_ps[:], iota_bf[:])

            # S-tile builders.  DVE: one is_equal.  ACT: Relu(1-|iota-dst|)
            # (exact for integer-valued inputs).  dcol indexes dst_sb/nds_sb.
            def build_s(spool, tpool, dcol, on_act):
                s_t = spool.tile([P, P], bf16, tag="s")
                if on_act:
                    u_t = tpool.tile([P, P], bf16, tag="u")
                    nc.scalar.activation(u_t[:], iota_bf[:], AF.Abs,
                                         bias=nds_sb[:, dcol:dcol + 1])
                    nc.scalar.activation(s_t[:], u_t[:], AF.Relu,
                                         bias=1.0, scale=-1.0)
                else:
                    nc.vector.tensor_scalar(
                        out=s_t[:], in0=iota_ps[:],
                        scalar1=dst_sb[:, dcol:dcol + 1], scalar2=None,
                        op0=OP.is_equal)
                return s_t

            # ---------- phase A: x @ W1 (+ dinv scale) ----
            with (
                tc.tile_pool(name="xk", bufs=1) as xpool,
                tc.tile_pool(name="psh1", bufs=2, space="PSUM") as ph1,
            ):
                xk_sb = []
                for k in range(fk):
                    xk = xpool.tile([P, nc_nodes], bf16, name=f"xk{k}")
                    nc.gpsimd.dma_start(out=xk[:], in_=xT_d[k * P:(k + 1) * P, :])
                    xk_sb.append(xk)

                # h1' = dinv * (x @ W1); kept in SBUF (self loops) and sent
                # to the all-gather bounce
                for w in range(nw):
                    r = rows_of(w)
                    h1p = ph1.tile([P, hdim], f32, tag="h1")
                    for k in range(fk):
                        nc.tensor.matmul(
                            h1p[:r, :],
                            xk_sb[k][:, w * P:w * P + r],
                            w1_bf[:, k * hdim:(k + 1) * hdim],
                            start=(k == 0), stop=(k == fk - 1))
                    nc.vector.tensor_scalar(
                        out=own_sb[:r, w * hdim:(w + 1) * hdim],
                        in0=h1p[:r, :],
                        scalar1=dinv_sb[:r, w:w + 1], scalar2=None,
                        op0=OP.mult)
                    nc.sync.dma_start(
                        out=ag1_in[w * P:w * P + r, :],
                        in_=own_sb[:r, w * hdim:w * hdim + hdim])

            nc.gpsimd.collective_compute(
                "AllGather", OP.bypass, replica_groups=rg,
                ins=[ag1_in.opt()], outs=[h1_tbl.opt()])
            if debug:
                nc.sync.dma_start(out=dbg_ag1[:, :], in_=ag1_in[:, :])
                nc.sync.dma_start(out=dbg_tbl[:, :], in_=h1_tbl[:, :])

            # ---------- aggregation pass helper ----------
            # Gathers + S-matmuls for one pass; epilogue(group, accs) runs
            # once per group with the list of (w, acc_psum).
            def agg_pass(tbl, mpool, spool, epool, pacc, epilogue, mbufs,
                         own, act_share=True):
                gi = 0
                qi = 0
                for g, gcalls in zip(groups, calls):
                    msg = mpool.tile([P, gmaxblk, P], bf16, tag="msg")
                    if gi < mbufs:  # stale-SBUF NaN guard on fresh slots
                        nc.vector.memset(msg[:], 0.0)
                    for (k, nidx, icol, blk0) in gcalls:
                        csz = min(cs, n - k * cs)
                        nc.gpsimd.dma_gather(
                            msg[:, blk0:blk0 + nidx // P, :],
                            tbl[k * cs:k * cs + csz, :],
                            src_sb[:, icol:icol + nidx // 16],
                            nidx, nidx, hdim, single_packet=False,
                            queue_num=qi % 4)
                        qi += 1
                    accs = []
                    for w in g:
                        blocks = wblocks[w]
                        acc = pacc.tile([P, hdim], f32, tag="acc")
                        # self loop: + own window rows (identity one-hot)
                        nc.tensor.matmul(acc[:], ident_bf[:],
                                         own[:, w * hdim:(w + 1) * hdim],
                                         start=True,
                                         stop=(len(blocks) == 0))
                        for j, (bpos, dcol) in enumerate(blocks):
                            # gathers keep POOL busy; POOL contends with DVE
                            # on the shared SBUF port - run 2/3 of the
                            # builds on ACT here
                            s_t = build_s(spool, spool, dcol,
                                          on_act=act_share and (j % 3 != 2))
                            nc.tensor.matmul(acc[:], s_t[:],
                                             msg[:, bpos, :],
                                             start=False,
                                             stop=(j == len(blocks) - 1))
                        accs.append((w, acc))
                    epilogue(g, accs)
                    gi += 1

            # ---------- phase B: aggregate layer 1, softplus, rescale --------
            with (
                tc.tile_pool(name="msgB", bufs=2) as mpool,
                tc.tile_pool(name="sB", bufs=6) as spool,
                tc.tile_pool(name="eB", bufs=3) as epool,
                tc.tile_pool(name="psB", bufs=4, space="PSUM") as pacc,
            ):
                def epi1(g, accs):
                    gw = len(accs)
                    stage = epool.tile([P, GROUP * hdim], f32, tag="st")
                    for i, (w, acc) in enumerate(accs):
                        # dinv*acc, PSUM -> staging (ACT, per window)
                        nc.scalar.activation(
                            stage[:, i * hdim:(i + 1) * hdim], acc[:],
                            AF.Identity, scale=dinv_sb[:, w:w + 1])
                    sl = stage[:, :gw * hdim]
                    nc.vector.tensor_add(sl, sl, b1_sb[:, :gw * hdim])
                    # softplus(x) = ln(exp(x) + 1), batched over the group;
                    # the +1 folds into Ln's bias
                    ex = epool.tile([P, GROUP * hdim], f32, tag="ex")
                    nc.scalar.activation(ex[:, :gw * hdim], sl, AF.Exp)
                    nc.scalar.activation(sl, ex[:, :gw * hdim], AF.Ln,
                                         bias=1.0)
                    for i, (w, acc) in enumerate(accs):
                        r = rows_of(w)
                        # hidden' = dinv*softplus; resident + all-gather copy
                        nc.scalar.activation(
                            own2_sb[:, w * hdim:(w + 1) * hdim],
                            stage[:, i * hdim:(i + 1) * hdim],
                            AF.Identity, scale=dinv_sb[:, w:w + 1])
                        nc.sync.dma_start(
                            out=ag2_in[w * P:w * P + r, :],
                            in_=own2_sb[:r, w * hdim:(w + 1) * hdim])

                agg_pass(h1_tbl, mpool, spool, epool, pacc, epi1, 2, own_sb)
            keepA.__exit__(None, None, None)

            nc.gpsimd.collective_compute(
                "AllGather", OP.bypass, replica_groups=rg,
                ins=[ag2_in.opt()], outs=[h2_tbl.opt()])
            if debug:
                nc.sync.dma_start(out=dbg_ag2[:, :], in_=ag2_in[:, :])
                nc.sync.dma_start(out=dbg_dinv[:, :], in_=dinv_sb[:])

            # ---------- phase C: aggregate layer 2, output heads -------------
            with (
                tc.tile_pool(name="zT", bufs=1) as zpool,
                tc.tile_pool(name="msgC", bufs=2) as mpool,
                tc.tile_pool(name="sC", bufs=6) as spool,
                tc.tile_pool(name="eC", bufs=4) as epool,
                tc.tile_pool(name="psC", bufs=3, space="PSUM") as pacc,
                tc.tile_pool(name="psT", bufs=2, space="PSUM") as ptr,
            ):
                zt_sb = zpool.tile([zc, nc_nodes], f32)

                def epi2(g, accs):
                    for i, (w, acc) in enumerate(accs):
                        r = rows_of(w)
                        a2bf = epool.tile([P, hdim], bf16, tag="a2bf")
                        nc.scalar.activation(a2bf[:], acc[:], AF.Identity,
                                             scale=dinv_sb[:, w:w + 1])
                        tp = ptr.tile([P, P], bf16, tag="tp")
                        nc.tensor.transpose(tp[:], a2bf[:], ident_bf[:])
                        a2t = epool.tile([P, P], bf16, tag="a2t")
                        nc.scalar.activation(a2t[:], tp[:], AF.Identity)
                        ztp = ptr.tile([zc, P], f32, tag="ztp")
                        nc.tensor.matmul(ztp[:], wc_bf[:], a2t[:],
                                         start=True, stop=True)
                        nc.scalar.activation(
                            zt_sb[0:zdim, w * P:w * P + r], ztp[0:zdim, :r],
                            AF.Identity, bias=bc_sb[0:zdim, 0:1])
                        nc.scalar.activation(
                            zt_sb[zdim:zc, w * P:w * P + r], ztp[zdim:zc, :r],
                            AF.Exp, bias=bc_sb[zdim:zc, 0:1])

                agg_pass(h2_tbl, mpool, spool, epool, pacc, epi2, 2, own2_sb)
                nc.sync.dma_start(out=out_d[:, :], in_=zt_sb[:])

    nc.compile()
    return nc


# --------------------------------------------------------------------------
# entry point
# --------------------------------------------------------------------------

def _install_profile_hook():
    """The agent image's antenv lacks axon_hooks; recreate it from the
    boot helpers so trace=True can capture NTFF exec times."""
    try:
        import antenv.axon_hooks  # noqa: F401
        return
    except ImportError:
        pass
    try:
        import types
        if "/root/.axon_site" not in sys.path:
            sys.path.insert(0, "/root/.axon_site")
        from trn_agent_boot.trn_boot import _ntff_profile_via_ctypes
        hook = _ntff_profile_via_ctypes("/opt/axon/libaxon_pjrt.so")
        mod = types.ModuleType("antenv.axon_hooks")
        mod.get_axon_ntff_profile_hook = lambda: hook
        mod.set_axon_ntff_profile_hook = lambda h: None
        sys.modules["antenv.axon_hooks"] = mod
    except Exception:
        pass


def kernel(x, edge_index, W1, b1, W_mu, b_mu, W_sig, b_sig, _trace=False,
           _debug=False):
    global LAST_RESULT
    if _trace:
        _install_profile_hook()
    x = np.ascontiguousarray(np.asarray(x, dtype=np.float32))
    W1 = np.asarray(W1, dtype=np.float32)
    b1 = np.asarray(b1, dtype=np.float32)
    W_mu = np.asarray(W_mu, dtype=np.float32)
    b_mu = np.asarray(b_mu, dtype=np.float32)
    W_sig = np.asarray(W_sig, dtype=np.float32)
    b_sig = np.asarray(b_sig, dtype=np.float32)

    n, fdim = x.shape
    hdim = W1.shape[1]
    zdim = W_mu.shape[1]
    dims = _host_prep(x, edge_index)
    nc_nodes = dims["nc_nodes"]

    nc = _build(dims, fdim, hdim, zdim, debug=_debug)

    wcat = np.ascontiguousarray(np.concatenate([W_mu, W_sig], axis=1))
    b1bc = np.ascontiguousarray(
        np.tile(b1[None, :], (P, GROUP)))
    bcat = np.ascontiguousarray(
        np.concatenate([b_mu, b_sig]).reshape(2 * zdim, 1))
    iota = np.ascontiguousarray(
        np.tile(np.arange(P, dtype=np.float32)[None, :], (P, 1)))
    ident = np.eye(P, dtype=np.float32)

    in_maps = []
    for c in range(NCORES):
        xt_c = np.ascontiguousarray(
            x[c * nc_nodes:(c + 1) * nc_nodes, :].T)
        in_maps.append({
            "xT": xt_c,
            "W1": W1, "Wcat": wcat, "b1bc": b1bc, "bcat": bcat,
            "iota": iota, "ident": ident,
            "srcidx": dims["src16"][c], "dstloc": dims["dstl"][c],
            "ndstloc": np.ascontiguousarray(-dims["dstl"][c]),
            "dinvt": dims["dinvt"][c],
        })

    res = run_bass_kernel_spmd(nc, in_maps, core_ids=list(range(NCORES)),
                               trace=_trace)
    LAST_RESULT = res

    z = np.concatenate([res.results[c]["z_out"] for c in range(NCORES)],
                       axis=1)  # [2Z, N]
    z_loc = np.ascontiguousarray(z[:zdim, :].T)
    z_scale = np.ascontiguousarray(z[zdim:, :].T)
    return z_loc, z_scale



# revision 20
# speedup vs baseline: 2.5606x; 1.3612x over previous
"""GCN encoder (VGAE-style) distributed Bass kernel for 8 TRN2 NeuronCores.

Math restructure (exact up to float reassociation):
  A_hat = D^-1/2 (A + I) D^-1/2  with  D = indegree(A)+1  (self loops)
  hidden = softplus(A_hat @ (x @ W1) + b1)
  agg    = A_hat @ hidden            # shared by both heads (aggregation is linear)
  z_loc  = agg @ W_mu + b_mu
  z_scale= exp(agg @ W_sig + b_sig)

Sharding: nodes (dim 0) across 8 cores; edges bucketed by destination core so
the scatter-add stays local; weights replicated; all-gather of the
dinv-scaled bf16 feature tables between layers.

Device mapping per core:
  - scatter-add via one-hot matmuls: S[e,d] = (dst_local[e]==d), accumulated
    on PE into PSUM per 128-node window.  S is built on DVE (is_equal vs an
    iota row) or on ACT (Relu(1-|iota-dst|)) - the split matters because the
    Q7 descriptor generation of the gathers contends with DVE on the shared
    POOL/DVE SBUF port.
  - per-edge source rows fetched from the all-gathered bf16 table with
    dma_gather (int16 indices -> the table is processed in <=32k-row chunks;
    edges are bucketed (dst window x src chunk) and 128-padded per bucket).
    ~8.3ns/row of serial Q7 descriptor generation is the kernel's floor.
  - self loops never touch the gather: they are one identity-matmul per
    window from the SBUF-resident scaled features.
  - epilogues (softplus=ln(1+exp), exp, deg^-1/2=exp(-0.5 ln)) on ScalarE,
    batched across window groups; a single ACT function table
    (natural_log_exp_and_others) covers Abs/Relu/Identity/Exp/Ln so only one
    table load is ever emitted (the stock per-function table pick would
    otherwise reload on every Ln<->Exp switch).
"""

import sys

import numpy as np

sys.path.insert(0, "/opt/trn_rl_repo")

import concourse.bass as bass
import concourse.bacc as bacc
import concourse.mybir as mybir
import concourse.tile as tile
from concourse.bass_utils import run_bass_kernel_spmd
from concourse.library_config import mlp

NCORES = 8
P = 128            # partitions / window size
CHUNK_MAX = 32000  # dma_gather int16 index reach
GROUP = 4          # windows per gather group / epilogue batch

LAST_RESULT = None  # BassKernelResults of the most recent run (test harness)

_ACT_TABLE = "natural_log_exp_and_others"
_act_patched = False


def _patch_act_tables():
    """Force every activation onto the one table that contains all of
    Abs/Relu/Identity/Exp/Ln.  The stock pass assigns each function its
    first-containing table, which thrashes table loads (~1.3us each) on
    every Ln<->Exp switch.  Emptying the other tables (order preserved, so
    act_func_set_id still matches act_info.json) makes the pass emit exactly
    one load."""
    global _act_patched
    if _act_patched:
        return
    import concourse.hw_specs as hw_specs
    orig = hw_specs.get_activation_tables

    def patched(arch):
        tabs = orig(arch)
        return {name: (fns if name == _ACT_TABLE else set())
                for name, fns in tabs.items()}

    bacc.get_activation_tables = patched
    _act_patched = True


# --------------------------------------------------------------------------
# host-side sharding / layout prep
# --------------------------------------------------------------------------

def _host_prep(x, edge_index):
    n = x.shape[0]
    assert n % NCORES == 0
    nc_nodes = n // NCORES
    nw = (nc_nodes + P - 1) // P

    nch = -(-n // CHUNK_MAX)           # table chunks
    cs = -(-n // nch)                  # chunk size (rows)
    assert cs <= 32767

    src_a = np.asarray(edge_index[0]).astype(np.int64)
    dst_a = np.asarray(edge_index[1]).astype(np.int64)
    # self loops are handled as identity matmuls on-device, not as edges

    core = dst_a // nc_nodes
    rem = dst_a - core * nc_nodes
    win = rem >> 7
    dloc = (rem & 127).astype(np.float32)
    chunk = src_a // cs
    bucket = (core * nw + win) * nch + chunk

    nb = NCORES * nw * nch
    order = np.argsort(bucket, kind="stable")
    sb = (src_a - chunk * cs)[order].astype(np.int16)  # chunk-local src idx
    db = dloc[order]
    counts = np.bincount(bucket, minlength=nb).reshape(NCORES, nw, nch)
    bstart = np.zeros(nb + 1, np.int64)
    bstart[1:] = np.cumsum(counts.reshape(-1))

    nblk = -(-counts.max(axis=0) // P)  # [nw, nch] shared across cores
    groups = [list(range(g, min(g + GROUP, nw))) for g in range(0, nw, GROUP)]
    maxnb = int(max(nblk.sum(axis=1)))  # most blocks any window owns

    # dinv = (indeg+1)^-1/2 — graph-structure metadata, host-computed like
    # the edge bucketing.  [NCORES, P, nw]: partition = row-in-window.
    deg = np.bincount(dst_a, minlength=n).astype(np.float32) + 1.0
    dinv_full = 1.0 / np.sqrt(deg)
    dinvt = np.ones((NCORES, nw * P), np.float32)
    for c in range(NCORES):
        dinvt[c, :nc_nodes] = dinv_full[c * nc_nodes:(c + 1) * nc_nodes]
    dinvt = np.ascontiguousarray(
        dinvt.reshape(NCORES, nw, P).transpose(0, 2, 1))

    # gather walk (chunk-major within group): src16 index layout + msg
    # block positions.  build walk (window-major within group): dstl
    # columns, so each window's S tiles build in ONE batched is_equal.
    calls = []        # per group: list of (k, num_idxs, idx_col0, blk0)
    gpos = {}         # (w, k) -> msg block position within its group
    idx_cols = 0
    seq_g = []        # gather-order walk: (w, k, nbk, icol)
    gmaxblk = 0
    for g in groups:
        gb = 0
        gcalls = []
        for k in range(nch):
            tot = int(sum(nblk[w, k] for w in g))
            if tot == 0:
                continue
            gcalls.append((k, tot * P, idx_cols, gb))
            for w in g:
                nbk = int(nblk[w, k])
                if nbk == 0:
                    continue
                gpos[(w, k)] = gb
                seq_g.append((w, k, nbk, idx_cols))
                idx_cols += nbk * 8   # 128 idx per block / 16 rows
                gb += nbk
        calls.append(gcalls)
        gmaxblk = max(gmaxblk, gb)

    wmeta = []        # per window: (dstl_col0, [msg block positions])
    dstl_cols = 0
    seq_b = []        # build-order walk: (w, k, nbk, dcol)
    for g in groups:
        for w in g:
            wcol0 = dstl_cols
            bpos = []
            for k in range(nch):
                nbk = int(nblk[w, k])
                if nbk == 0:
                    continue
                seq_b.append((w, k, nbk, dstl_cols))
                for b in range(nbk):
                    bpos.append(gpos[(w, k)] + b)
                dstl_cols += nbk
            wmeta.append((wcol0, bpos))

    ltot = max(idx_cols, 1)
    btot = max(dstl_cols, 1)
    src16 = np.zeros((NCORES, P, ltot), np.int16)
    dstl = np.full((NCORES, P, btot), -1.0, np.float32)
    for c in range(NCORES):
        for (w, k, nbk, icol) in seq_g:
            b = (c * nw + w) * nch + k
            s, e = bstart[b], bstart[b + 1]
            cnt = int(e - s)
            sl = np.zeros(nbk * P, np.int16)
            sl[:cnt] = sb[s:e]
            # stream position i -> idx16[i%16, i//16]; replicated x8 rows
            src16[c][:, icol:icol + nbk * 8] = np.tile(
                sl.reshape(nbk * 8, 16).T, (8, 1))
        for (w, k, nbk, dcol) in seq_b:
            b = (c * nw + w) * nch + k
            s, e = bstart[b], bstart[b + 1]
            cnt = int(e - s)
            dl = np.full(nbk * P, -1.0, np.float32)
            dl[:cnt] = db[s:e]
            # edge i -> msg[partition i%128, block i//128]
            dstl[c][:, dcol:dcol + nbk] = dl.reshape(nbk, P).T
    return {
        "n": n, "nc_nodes": nc_nodes, "nw": nw, "nch": nch, "cs": cs,
        "groups": groups, "calls": calls, "wmeta": wmeta, "maxnb": maxnb,
        "gmaxblk": gmaxblk, "ltot": ltot, "btot": btot,
        "src16": src16, "dstl": dstl, "dinvt": dinvt,
    }


# --------------------------------------------------------------------------
# device program
# --------------------------------------------------------------------------

def _build(dims, fdim, hdim, zdim, debug=False):
    """Build the SPMD Bass program (identical for all 8 cores)."""
    n = dims["n"]
    nc_nodes = dims["nc_nodes"]
    nw = dims["nw"]
    nch = dims["nch"]
    cs = dims["cs"]
    groups = dims["groups"]
    calls = dims["calls"]
    wmeta = dims["wmeta"]
    maxnb = dims["maxnb"]
    gmaxblk = dims["gmaxblk"]
    ltot = dims["ltot"]
    btot = dims["btot"]
    fk = fdim // P
    zc = 2 * zdim
    assert hdim == P and zc == P

    f32 = mybir.dt.float32
    bf16 = mybir.dt.bfloat16
    i16 = mybir.dt.int16
    AF = mybir.ActivationFunctionType
    OP = mybir.AluOpType
    rg = [list(range(NCORES))]

    _patch_act_tables()
    nc = bacc.Bacc("TRN2", target_bir_lowering=False, debug=False,
                   num_devices=NCORES, num_swdge_queues=4)

    # ---- kernel I/O ----
    xT_d = nc.dram_tensor("xT", [fdim, nc_nodes], f32, kind="ExternalInput")
    w1_d = nc.dram_tensor("W1", [fdim, hdim], f32, kind="ExternalInput")
    wc_d = nc.dram_tensor("Wcat", [hdim, zc], f32, kind="ExternalInput")
    b1_d = nc.dram_tensor("b1bc", [P, GROUP * hdim], f32, kind="ExternalInput")
    bc_d = nc.dram_tensor("bcat", [zc, 1], f32, kind="ExternalInput")
    id_d = nc.dram_tensor("ident", [P, P], f32, kind="ExternalInput")
    iw_d = nc.dram_tensor("iotaw", [P, maxnb * P], f32, kind="ExternalInput")
    src_d = nc.dram_tensor("srcidx", [P, ltot], i16, kind="ExternalInput")
    dst_d = nc.dram_tensor("dstloc", [P, btot], f32, kind="ExternalInput")
    dnv_d = nc.dram_tensor("dinvt", [P, nw], f32, kind="ExternalInput")
    out_d = nc.dram_tensor("z_out", [zc, nc_nodes], f32, kind="ExternalOutput")
    if debug:
        dbg_dinv = nc.dram_tensor("dbg_dinv", [P, nw], f32,
                                  kind="ExternalOutput")
        dbg_ag1 = nc.dram_tensor("dbg_ag1", [nc_nodes, hdim], bf16,
                                 kind="ExternalOutput")
        dbg_tbl = nc.dram_tensor("dbg_tbl", [n, hdim], bf16,
                                 kind="ExternalOutput")
        dbg_ag2 = nc.dram_tensor("dbg_ag2", [nc_nodes, hdim], bf16,
                                 kind="ExternalOutput")

    def rows_of(w):
        return min(P, nc_nodes - w * P)

    with tile.TileContext(nc) as tc:
        with (
            tc.tile_pool(name="const", bufs=1) as cpool,
            tc.tile_pool(name="idx", bufs=1) as ipool,
            tc.tile_pool(name="keep", bufs=1) as kpool,
            tc.tile_pool(name="dram", bufs=1, space="DRAM") as dpool,
        ):
            nc.gpsimd.load_library(mlp)

            ident_bf = cpool.tile([P, P], bf16)
            iow_sb = cpool.tile([P, maxnb, P], bf16, name="iotaw")
            w1_bf = cpool.tile([P, fk * hdim], bf16, name="w1bf")
            wc_bf = cpool.tile([P, zc], bf16)
            b1_sb = cpool.tile([P, GROUP * hdim], f32)
            bc_sb = cpool.tile([zc, 1], f32)
            dinv_sb = cpool.tile([P, nw], f32)

            src_sb = ipool.tile([P, ltot], i16)
            dst_sb = ipool.tile([P, btot], bf16)
            # dinv-scaled own rows (self-loop source): own2 = hidden' layer
            own2_sb = kpool.tile([P, nw * hdim], bf16, name="own2")
            nc.vector.memset(own2_sb[:], 0.0)

            nc.gpsimd.dma_start(out=ident_bf[:], in_=id_d[:, :])
            nc.gpsimd.dma_start(out=iow_sb[:, :, :], in_=iw_d[:, :])
            for k in range(fk):
                nc.gpsimd.dma_start(
                    out=w1_bf[:, k * hdim:(k + 1) * hdim],
                    in_=w1_d[k * P:(k + 1) * P, :])
            nc.gpsimd.dma_start(out=wc_bf[:], in_=wc_d[:, :])
            nc.gpsimd.dma_start(out=dst_sb[:], in_=dst_d[:, :])
            nc.sync.dma_start(out=b1_sb[:], in_=b1_d[:, :])
            nc.sync.dma_start(out=bc_sb[:], in_=bc_d[:, :])
            nc.sync.dma_start(out=src_sb[:], in_=src_d[:, :])
            nc.sync.dma_start(out=dinv_sb[:], in_=dnv_d[:, :])

            ag1_in = dpool.tile([nc_nodes, hdim], bf16)
            ag2_in = dpool.tile([nc_nodes, hdim], bf16)
            h1_tbl = dpool.tile([n, hdim], bf16, addr_space="Shared")
            h2_tbl = dpool.tile([n, hdim], bf16, addr_space="Shared")

            # own1 = h1' layer self-loop source; lives until end of phase B
            keepA = tc.tile_pool(name="keepA", bufs=1)
            keepA_pool = keepA.__enter__()
            own_sb = keepA_pool.tile([P, nw * hdim], bf16, name="own1")
            nc.vector.memset(own_sb[:], 0.0)

            # ---------- phase A: x @ W1 (+ dinv scale) ----
            with (
                tc.tile_pool(name="xk", bufs=1) as xpool,
                tc.tile_pool(name="psh1", bufs=2, space="PSUM") as ph1,
            ):
                xk_sb = []
                for k in range(fk):
                    xk = xpool.tile([P, nc_nodes], bf16, name=f"xk{k}")
                    nc.gpsimd.dma_start(out=xk[:], in_=xT_d[k * P:(k + 1) * P, :])
                    xk_sb.append(xk)

                # h1' = dinv * (x @ W1); kept in SBUF (self loops) and sent
                # to the all-gather bounce
                for w in range(nw):
                    r = rows_of(w)
                    h1p = ph1.tile([P, hdim], f32, tag="h1")
                    for k in range(fk):
                        nc.tensor.matmul(
                            h1p[:r, :],
                            xk_sb[k][:, w * P:w * P + r],
                            w1_bf[:, k * hdim:(k + 1) * hdim],
                            start=(k == 0), stop=(k == fk - 1))
                    nc.vector.tensor_scalar(
                        out=own_sb[:r, w * hdim:(w + 1) * hdim],
                        in0=h1p[:r, :],
                        scalar1=dinv_sb[:r, w:w + 1], scalar2=None,
                        op0=OP.mult)
                    nc.sync.dma_start(
                        out=ag1_in[w * P:w * P + r, :],
                        in_=own_sb[:r, w * hdim:w * hdim + hdim])

            nc.gpsimd.collective_compute(
                "AllGather", OP.bypass, replica_groups=rg,
                ins=[ag1_in.opt()], outs=[h1_tbl.opt()])
            if debug:
                nc.sync.dma_start(out=dbg_ag1[:, :], in_=ag1_in[:, :])
                nc.sync.dma_start(out=dbg_tbl[:, :], in_=h1_tbl[:, :])

            # ---------- aggregation pass helper ----------
            # Gathers + S-matmuls for one pass; epilogue(group, accs) runs
            # once per group with the list of (w, acc_psum).  All S tiles of
            # a window build in ONE batched DVE is_equal (bf16 in/out).
            def agg_pass(tbl, mpool, spool, epool, pacc, epilogue, mbufs,
                         own):
                gi = 0
                qi = 0
                for g, gcalls in zip(groups, calls):
                    msg = mpool.tile([P, gmaxblk, P], bf16, tag="msg")
                    if gi < mbufs:  # stale-SBUF NaN guard on fresh slots
                        nc.vector.memset(msg[:], 0.0)
                    for (k, nidx, icol, blk0) in gcalls:
                        csz = min(cs, n - k * cs)
                        nc.gpsimd.dma_gather(
                            msg[:, blk0:blk0 + nidx // P, :],
                            tbl[k * cs:k * cs + csz, :],
                            src_sb[:, icol:icol + nidx // 16],
                            nidx, nidx, hdim, single_packet=False,
                            queue_num=qi % 4)
                        qi += 1
                    accs = []
                    for w in g:
                        wcol0, bpos = wmeta[w]
                        nbw = len(bpos)
                        acc = pacc.tile([P, hdim], f32, tag="acc")
                        # self loop: + own window rows (identity one-hot)
                        nc.tensor.matmul(acc[:], ident_bf[:],
                                         own[:, w * hdim:(w + 1) * hdim],
                                         start=True,
                                         stop=(nbw == 0))
                        if nbw:
                            s_w = spool.tile([P, maxnb, P], bf16, tag="s")
                            nc.vector.tensor_tensor(
                                out=s_w[:, 0:nbw, :],
                                in0=iow_sb[:, 0:nbw, :],
                                in1=dst_sb[:, wcol0:wcol0 + nbw]
                                    .to_broadcast([P, nbw, P]),
                                op=OP.is_equal)
                        for j, bp in enumerate(bpos):
                            nc.tensor.matmul(acc[:], s_w[:, j, :],
                                             msg[:, bp, :],
                                             start=False,
                                             stop=(j == nbw - 1))
                        accs.append((w, acc))
                    epilogue(g, accs)
                    gi += 1

            # ---------- phase B: aggregate layer 1, softplus, rescale --------
            with (
                tc.tile_pool(name="msgB", bufs=2) as mpool,
                tc.tile_pool(name="sB", bufs=3) as spool,
                tc.tile_pool(name="eB", bufs=3) as epool,
                tc.tile_pool(name="psB", bufs=4, space="PSUM") as pacc,
            ):
                def epi1(g, accs):
                    gw = len(accs)
                    stage = epool.tile([P, GROUP * hdim], f32, tag="st")
                    for i, (w, acc) in enumerate(accs):
                        # dinv*acc, PSUM -> staging (ACT, per window)
                        nc.scalar.activation(
                            stage[:, i * hdim:(i + 1) * hdim], acc[:],
                            AF.Identity, scale=dinv_sb[:, w:w + 1])
                    sl = stage[:, :gw * hdim]
                    nc.vector.tensor_add(sl, sl, b1_sb[:, :gw * hdim])
                    # softplus(x) = ln(exp(x) + 1), batched over the group;
                    # the +1 folds into Ln's bias
                    ex = epool.tile([P, GROUP * hdim], f32, tag="ex")
                    nc.scalar.activation(ex[:, :gw * hdim], sl, AF.Exp)
                    nc.scalar.activation(sl, ex[:, :gw * hdim], AF.Ln,
                                         bias=1.0)
                    for i, (w, acc) in enumerate(accs):
                        r = rows_of(w)
                        # hidden' = dinv*softplus; resident + all-gather copy
                        nc.scalar.activation(
                            own2_sb[:, w * hdim:(w + 1) * hdim],
                            stage[:, i * hdim:(i + 1) * hdim],
                            AF.Identity, scale=dinv_sb[:, w:w + 1])
                        nc.sync.dma_start(
                            out=ag2_in[w * P:w * P + r, :],
                            in_=own2_sb[:r, w * hdim:(w + 1) * hdim])

                agg_pass(h1_tbl, mpool, spool, epool, pacc, epi1, 2, own_sb)
            keepA.__exit__(None, None, None)

            nc.gpsimd.collective_compute(
                "AllGather", OP.bypass, replica_groups=rg,
                ins=[ag2_in.opt()], outs=[h2_tbl.opt()])
            if debug:
                nc.sync.dma_start(out=dbg_ag2[:, :], in_=ag2_in[:, :])
                nc.sync.dma_start(out=dbg_dinv[:, :], in_=dinv_sb[:])

            # ---------- phase C: aggregate layer 2, output heads -------------
            with (
                tc.tile_pool(name="zT", bufs=1) as zpool,
                tc.tile_pool(name="msgC", bufs=2) as mpool,
                tc.tile_pool(name="sC", bufs=3) as spool,
                tc.tile_pool(name="eC", bufs=4) as epool,
                tc.tile_pool(name="psC", bufs=3, space="PSUM") as pacc,
                tc.tile_pool(name="psT", bufs=2, space="PSUM") as ptr,
            ):
                zt_sb = zpool.tile([zc, nc_nodes], f32)

                def epi2(g, accs):
                    for i, (w, acc) in enumerate(accs):
                        r = rows_of(w)
                        a2bf = epool.tile([P, hdim], bf16, tag="a2bf")
                        nc.scalar.activation(a2bf[:], acc[:], AF.Identity,
                                             scale=dinv_sb[:, w:w + 1])
                        tp = ptr.tile([P, P], bf16, tag="tp")
                        nc.tensor.transpose(tp[:], a2bf[:], ident_bf[:])
                        a2t = epool.tile([P, P], bf16, tag="a2t")
                        nc.scalar.activation(a2t[:], tp[:], AF.Identity)
                        ztp = ptr.tile([zc, P], f32, tag="ztp")
                        nc.tensor.matmul(ztp[:], wc_bf[:], a2t[:],
                                         start=True, stop=True)
                        nc.scalar.activation(
                            zt_sb[0:zdim, w * P:w * P + r], ztp[0:zdim, :r],
                            AF.Identity, bias=bc_sb[0:zdim, 0:1])
                        nc.scalar.activation(
                            zt_sb[zdim:zc, w * P:w * P + r], ztp[zdim:zc, :r],
                            AF.Exp, bias=bc_sb[zdim:zc, 0:1])

                agg_pass(h2_tbl, mpool, spool, epool, pacc, epi2, 2, own2_sb)
                nc.sync.dma_start(out=out_d[:, :], in_=zt_sb[:])

    nc.compile()
    return nc


# --------------------------------------------------------------------------
# entry point
# --------------------------------------------------------------------------

def _install_profile_hook():
    """The agent image's antenv lacks axon_hooks; recreate it from the
    boot helpers so trace=True can capture NTFF exec times."""
    try:
        import antenv.axon_hooks  # noqa: F401
        return
    except ImportError:
        pass
    try:
        import types
        if "/root/.axon_site" not in sys.path:
            sys.path.insert(0, "/root/.axon_site")
        from trn_agent_boot.trn_boot import _ntff_profile_via_ctypes
        hook = _ntff_profile_via_ctypes("/opt/axon/libaxon_pjrt.so")
        mod = types.ModuleType("antenv.axon_hooks")
        mod.get_axon_ntff_profile_hook = lambda: hook
        mod.set_axon_ntff_profile_hook = lambda h: None
        sys.modules["antenv.axon_hooks"] = mod
    except Exception:
        pass


def kernel(x, edge_index, W1, b1, W_mu, b_mu, W_sig, b_sig, _trace=False,
           _debug=False):
    global LAST_RESULT
    if _trace:
        _install_profile_hook()
    x = np.ascontiguousarray(np.asarray(x, dtype=np.float32))
    W1 = np.asarray(W1, dtype=np.float32)
    b1 = np.asarray(b1, dtype=np.float32)
    W_mu = np.asarray(W_mu, dtype=np.float32)
    b_mu = np.asarray(b_mu, dtype=np.float32)
    W_sig = np.asarray(W_sig, dtype=np.float32)
    b_sig = np.asarray(b_sig, dtype=np.float32)

    n, fdim = x.shape
    hdim = W1.shape[1]
    zdim = W_mu.shape[1]
    dims = _host_prep(x, edge_index)
    nc_nodes = dims["nc_nodes"]

    nc = _build(dims, fdim, hdim, zdim, debug=_debug)

    wcat = np.ascontiguousarray(np.concatenate([W_mu, W_sig], axis=1))
    b1bc = np.ascontiguousarray(
        np.tile(b1[None, :], (P, GROUP)))
    bcat = np.ascontiguousarray(
        np.concatenate([b_mu, b_sig]).reshape(2 * zdim, 1))
    iotaw = np.ascontiguousarray(
        np.tile(np.arange(P, dtype=np.float32)[None, :],
                (P, dims["maxnb"])))
    ident = np.eye(P, dtype=np.float32)

    in_maps = []
    for c in range(NCORES):
        xt_c = np.ascontiguousarray(
            x[c * nc_nodes:(c + 1) * nc_nodes, :].T)
        in_maps.append({
            "xT": xt_c,
            "W1": W1, "Wcat": wcat, "b1bc": b1bc, "bcat": bcat,
            "iotaw": iotaw, "ident": ident,
            "srcidx": dims["src16"][c], "dstloc": dims["dstl"][c],
            "dinvt": dims["dinvt"][c],
        })

    res = run_bass_kernel_spmd(nc, in_maps, core_ids=list(range(NCORES)),
                               trace=_trace)
    LAST_RESULT = res

    z = np.concatenate([res.results[c]["z_out"] for c in range(NCORES)],
                       axis=1)  # [2Z, N]
    z_loc = np.ascontiguousarray(z[:zdim, :].T)
    z_scale = np.ascontiguousarray(z[zdim:, :].T)
    return z_loc, z_scale

